# revision 1
# baseline (speedup 1.0000x reference)
"""BitNet ternary linear (nn_BitNetLinear4Bit) Trainium2 Bass kernel.

out = x @ (alpha * clip(round(w/alpha), -1, 1))^T + bias
  x: [2, 2048, 4096] f32, w: [11008, 4096] f32, alpha: [1] f32, bias: [11008] f32
  -> out: [2, 2048, 11008] f32

Sharding: column-parallel over 8 cores. Each core gets the full x
(replicated) and a 1376-row slice of w / bias; it produces a
[4096, 1376] slice of the output which the host concatenates.

Per-core algorithm (all math on device):
  Phase W: stream w-shard 128-row x 1024-col blocks, ternarize
    (t = (w >= a/2) - (w <= -a/2), exact in bf16; equals
    clip(round(w/a),-1,1) away from the measure-zero half-integer
    boundary; compares on GpSimd, combine on DVE), XBAR-transpose
    directly into resident transposed weights wtg[g] [128, CC, 32, 128]
    bf16 — the [P, chunk, KO, P] layout makes every XBAR destination a
    contiguous block while the matmul reads the strided rhs
    wtg[g][:, :, ko, :].
  Phase MM: per 128-token block: DMA x rows (contiguous f32), DVE-cast
    to bf16, XBAR-transpose to xt [128, 32, 128]; per output group
    accumulate K=4096 with 32 bf16 matmuls (N<=512) into one PSUM bank;
    evict ACT copy*alpha then GpSimd +bias; DMA out. Matmul groups
    unlock as soon as phase W finishes each weight group so the PE
    starts ~1/3 into phase W.

alpha is read on the host and baked into the program as an immediate;
the compiled program is cached keyed on alpha and recompiled if it
changes.
"""

import numpy as np

B, S, DIN, DOUT = 2, 2048, 4096, 11008
NCORES = 8
DOUT_SH = DOUT // NCORES  # 1376
TOK = B * S  # 4096
P = 128


def _build(alpha_f, TOK=TOK, DIN=DIN, DOUT_SH=DOUT_SH, debug=False):
    import concourse.mybir as mybir
    from concourse import bacc
    from concourse.tile import TileContext

    f32 = mybir.dt.float32
    bf16 = mybir.dt.bfloat16
    Alu = mybir.AluOpType
    Act = mybir.ActivationFunctionType

    KO = DIN // P
    M_SUBS = TOK // P
    W_CHUNKS = (DOUT_SH + P - 1) // P  # 11 (last chunk 96 rows, zero-padded)
    QCOL = min(1024, DIN)
    QK = QCOL // P  # ko levels per quantize chunk
    # output groups: one psum bank each, up to 4 chunks (<=512 cols incl pad)
    GROUPS = []  # (first chunk, n chunks, dout start, real width)
    c = 0
    while c < W_CHUNKS:
        cc = min(4, W_CHUNKS - c)
        width = min(DOUT_SH, (c + cc) * P) - c * P  # real (unpadded) width
        GROUPS.append((c, cc, c * P, width))
        c += cc

    a2 = float(alpha_f) * 0.5

    nc = bacc.Bacc(None, target_bir_lowering=False, debug=debug)
    x_d = nc.dram_tensor("x", [TOK, DIN], f32, kind="ExternalInput")
    w_d = nc.dram_tensor("w", [DOUT_SH, DIN], f32, kind="ExternalInput")
    nc.dram_tensor("alpha", [1], f32, kind="ExternalInput")
    b_d = nc.dram_tensor("bias", [DOUT_SH], f32, kind="ExternalInput")
    o_d = nc.dram_tensor("out", [TOK, DOUT_SH], f32, kind="ExternalOutput")

    from concourse.masks import make_identity

    with TileContext(nc) as tc:
        with (
            tc.tile_pool(name="const", bufs=1) as const,
            tc.tile_pool(name="wres", bufs=1) as wres,
            tc.tile_pool(name="ptp", bufs=2, space="PSUM") as ptp,
        ):
            ident = const.tile([P, P], bf16)
            make_identity(nc, ident)
            bias_sb = const.tile([P, DOUT_SH], f32)
            nc.sync.dma_start(
                bias_sb[:],
                b_d[:].rearrange("(a n) -> a n", a=1).to_broadcast((P, DOUT_SH)),
            )

            # resident transposed ternary weights, one tensor per output
            # group, chunk-major so XBAR destinations are contiguous:
            # wtg[g][p, i, ko, j] = t[(c0+i)*128 + j, ko*128 + p]
            wtg = [
                wres.tile([P, cc, KO, P], bf16, name=f"wtg_{g}")
                for g, (_, cc, _, _) in enumerate(GROUPS)
            ]

            # ---- Phase W: quantize + transpose w shard ----
            with tc.tile_pool(name="wq", bufs=4) as wq:
                for g, (c0, cc, n0, width) in enumerate(GROUPS):
                    for i in range(cc):
                        c = c0 + i
                        rc = min(P, DOUT_SH - c * P)  # 128 or 96 (last)
                        for q in range(DIN // QCOL):
                            wrow = wq.tile([P, QCOL], f32, tag="wrow")
                            if rc < P:
                                nc.gpsimd.memset(wrow[:], 0.0)
                            nc.sync.dma_start(
                                wrow[:rc, :],
                                w_d[c * P : c * P + rc, q * QCOL : (q + 1) * QCOL],
                            )
                            # t = (w >= a/2) - (w <= -a/2) in {-1,0,1}
                            le = wq.tile([P, QCOL], bf16, tag="le")
                            nc.vector.tensor_scalar(
                                le[:], wrow[:], -a2, None, Alu.is_le
                            )
                            tq = wq.tile([P, QCOL], bf16, tag="tq")
                            nc.vector.scalar_tensor_tensor(
                                tq[:], wrow[:], a2, le[:], Alu.is_ge, Alu.subtract
                            )
                            # PE-transpose each 128x128 block into the
                            # chunk-contiguous region (zero-padded rows of
                            # the last chunk land in cols >= rc = zeros)
                            for bb in range(QK):
                                pt = ptp.tile([P, P], bf16, tag="pt")
                                nc.tensor.transpose(
                                    pt[:], tq[:, bb * P : (bb + 1) * P], ident[:]
                                )
                                nc.any.tensor_copy(
                                    wtg[g][:, i, q * QK + bb, :], pt[:]
                                )

            # ---- Phase MM ----
            with (
                tc.tile_pool(name="xp", bufs=2) as xp,
                tc.tile_pool(name="xtp", bufs=3) as xtp,
                tc.tile_pool(name="op", bufs=4) as op,
                tc.tile_pool(name="pso", bufs=6, space="PSUM") as pso,
            ):
                for ms in range(M_SUBS):
                    xbf = xp.tile([P, DIN], bf16, tag="xbf")
                    for h in range(4):
                        hw = DIN // 4
                        xrow = xp.tile([P, hw], f32, tag="xrow")
                        nc.sync.dma_start(
                            xrow[:], x_d[ms * P : (ms + 1) * P, h * hw : (h + 1) * hw]
                        )
                        nc.vector.tensor_copy(xbf[:, h * hw : (h + 1) * hw], xrow[:])
                    xt = xtp.tile([P, KO, P], bf16, tag="xt")
                    nc.sync.dma_start_transpose(xt[:], xbf[:])

                    for g, (c0, cc, n0, width) in enumerate(GROUPS):
                        po = pso.tile([P, 512], f32, tag="po", name=f"po_{ms}_{g}")
                        pw = cc * P  # padded width (>= real width)
                        for ko in range(KO):
                            nc.tensor.matmul(
                                po[:, :pw],
                                xt[:, ko, :],
                                wtg[g][:, :, ko, :],
                                start=(ko == 0),
                                stop=(ko == KO - 1),
                            )
                        # out = psum * alpha (ACT), then += bias (GpSimd)
                        osb = op.tile([P, 512], f32, tag="osb", name=f"osb_{ms}_{g}")
                        nc.scalar.activation(
                            osb[:, :width],
                            po[:, :width],
                            Act.Copy,
                            scale=float(alpha_f),
                        )
                        nc.gpsimd.tensor_add(
                            osb[:, :width],
                            osb[:, :width],
                            bias_sb[:, n0 : n0 + width],
                        )
                        nc.sync.dma_start(
                            o_d[ms * P : (ms + 1) * P, n0 : n0 + width],
                            osb[:, :width],
                        )

    nc.compile()
    return nc


_CACHE = {}


def _get_nc(alpha_f):
    key = float(alpha_f)
    if key not in _CACHE:
        _CACHE[key] = _build(key)
    return _CACHE[key]


def kernel(x, w, alpha, bias):
    from concourse.bass_utils import run_bass_kernel_spmd

    alpha2 = np.ascontiguousarray(np.asarray(alpha, dtype=np.float32).reshape(1))
    nc = _get_nc(alpha2[0])
    x2 = np.ascontiguousarray(np.asarray(x, dtype=np.float32).reshape(TOK, DIN))
    in_maps = []
    for c in range(NCORES):
        in_maps.append(
            {
                "x": x2,
                "w": np.ascontiguousarray(w[c * DOUT_SH : (c + 1) * DOUT_SH]),
                "alpha": alpha2,
                "bias": np.ascontiguousarray(bias[c * DOUT_SH : (c + 1) * DOUT_SH]),
            }
        )
    res = run_bass_kernel_spmd(nc, in_maps, core_ids=list(range(NCORES)))
    outs = [res.results[c]["out"] for c in range(NCORES)]
    out = np.concatenate(outs, axis=1).reshape(B, S, DOUT)
    return np.ascontiguousarray(out.astype(np.float32))

